# revision 1
# baseline (speedup 1.0000x reference)
"""Distributed multi-head self-attention for Trainium2 (8 NeuronCores).

Problem: b=4, n=2048, dim=1024, heads=16, dim_head=64.
  q = x@Wq; k,v = split(x@Wkv, 2); out = softmax(q k^T / 8) v; y = out@Wout + bout

Sharding: core c <-> (batch b=c//2, head-group g=c%2). Each core computes
q/k/v + attention for its batch's 8 heads (tensor-parallel columns of
Wq/Wkv). The pair (b,0)/(b,1) AllGathers the transposed bf16 attention
outputs (per head-pair, overlapped with attention compute; the last pair
streams per-i-chunk), then each core runs the output projection with the
full Wout over ITS HALF of the sequence (selected from the gathered buffer
with per-core one-hot mask inputs, since the SPMD graph is identical on all
cores). Core 2b+g emits out rows [1024g : 1024(g+1)] of batch b; the host
reassembles [4, 2048, 1024].

TensorEngine math is bf16 with f32 PSUM accumulation. Softmax skips
max-subtraction (scaled scores are ~N(0,1)); exp runs on the scalar engine
(PSUM f32 in -> bf16 SBUF out, scale fused). Denominators come from a ones
column appended to v; the PSUM accumulator is released with two fast copies
and normalization (reciprocal_approx_fast + partition_broadcast + in-place
multiply) runs lazily off the critical path. Score matmuls (K=64) run two
heads concurrently via tile_position row groups. q/k projections for pair
p+1 are emitted after attention(p) and v projections inside attention(0)'s
first column loop, so the TensorEngine fills its slack while attention is
ACT(exp)-bound.
"""

import numpy as np

import concourse.mybir as mybir
import concourse.tile as tile
from concourse import bacc, bass_utils
from concourse.masks import make_identity

N_CORES = 8
B, N, D = 4, 2048, 1024
GH = 8          # heads per core
DH = 64
IN = GH * DH    # 512 inner dims per core
SCALE = DH ** -0.5
PT = 128
KD = D // PT    # 8 dim tiles
MS = N // PT    # 16 seq tiles
MI = IN // PT   # 4 head-pair tiles per core
NH = N // 2     # out rows per core
F32 = mybir.dt.float32
BF16 = mybir.dt.bfloat16
RG = [[0, 1], [2, 3], [4, 5], [6, 7]]

_COMPILED = None


def build():
    nc = bacc.Bacc("TRN2", target_bir_lowering=False, debug=False, num_devices=N_CORES)

    x_ext = nc.dram_tensor("x", [N, D], F32, kind="ExternalInput")
    wq_ext = nc.dram_tensor("wq", [D, IN], F32, kind="ExternalInput")
    wk_ext = nc.dram_tensor("wk", [D, IN], F32, kind="ExternalInput")
    wv_ext = nc.dram_tensor("wv", [D, IN], F32, kind="ExternalInput")
    wout_ext = nc.dram_tensor("wout", [D, D], F32, kind="ExternalInput")
    bout_ext = nc.dram_tensor("bout", [D], F32, kind="ExternalInput")
    sel_ext = nc.dram_tensor("sel", [1, 2], F32, kind="ExternalInput")
    out_ext = nc.dram_tensor("out", [NH, D], F32, kind="ExternalOutput")

    with tile.TileContext(nc) as tc:
        with (
            tc.tile_pool(name="const", bufs=1) as constp,
            tc.tile_pool(name="wpool", bufs=1) as wpool,
            tc.tile_pool(name="qkv", bufs=1) as qkv,
            tc.tile_pool(name="attout", bufs=1) as attoutp,
            tc.tile_pool(name="dram", bufs=1, space="DRAM") as dram,
        ):
            ident = constp.tile([PT, PT], BF16)
            make_identity(nc, ident[:])
            bias_row = constp.tile([1, D], F32)
            nc.sync.dma_start(bias_row[:], bout_ext[None, :])
            bias_bf = constp.tile([1, D], BF16)
            nc.vector.tensor_copy(bias_bf[:], bias_row[:])
            ones_col = constp.tile([1, PT], BF16)
            nc.gpsimd.memset(ones_col[:], 1.0)
            sel_row = constp.tile([1, 2], F32)
            nc.sync.dma_start(sel_row[:], sel_ext[:])
            s0_bc = constp.tile([PT, 1], F32)
            s1_bc = constp.tile([PT, 1], F32)
            nc.gpsimd.partition_broadcast(s0_bc[:], sel_row[:, 0:1])
            nc.gpsimd.partition_broadcast(s1_bc[:], sel_row[:, 1:2])

            wq_bf = [wpool.tile([PT, IN], BF16, name=f"wq_bf{k}") for k in range(KD)]
            wk_bf = [wpool.tile([PT, IN], BF16, name=f"wk_bf{k}") for k in range(KD)]
            wo_bf = [wpool.tile([PT, D], BF16, name=f"wo_bf{k}") for k in range(KD)]

            qT = [qkv.tile([PT, N], BF16, name=f"qT{m}") for m in range(MI)]
            kT = [qkv.tile([PT, N], BF16, name=f"kT{m}") for m in range(MI)]
            vsb = [qkv.tile([PT, GH, 66], BF16, name=f"v{s}") for s in range(MS)]

            attoutT = [attoutp.tile([PT, N], BF16, name=f"attoutT{p}") for p in range(MI)]
            # after AG(p) the attoutT data is snapshotted to DRAM; reuse the
            # tile halves for the mask-selected gathered k-tiles kk=p
            # (cols 0:NH) and kk=p+MI (cols NH:N)
            attThalf = [
                attoutT[k % MI][:, (k // MI) * NH:(k // MI + 1) * NH]
                for k in range(KD)
            ]
            ag_in = [dram.tile([PT, N], BF16, name=f"ag_in{p}") for p in range(MI)]
            ag_out = [dram.tile([2 * PT, N], BF16, name=f"ag_out{p}") for p in range(MI)]
            ag_chunk = [dram.tile([2 * PT, 512], BF16, name=f"ag_chunk{i}") for i in range(4)]
            ag_cin = [dram.tile([PT, 512], BF16, name=f"ag_cin{i}") for i in range(4)]

            # ================= phase 0 ==============
            with (
                tc.tile_pool(name="xT", bufs=1) as xTp,
                tc.tile_pool(name="stage", bufs=3) as stage,
                tc.tile_pool(name="wstage", bufs=3) as wstage,
                tc.tile_pool(name="xbf", bufs=2) as xbfp,
                tc.tile_pool(name="wvp", bufs=1) as wvp,
            ):
                xT = [xTp.tile([PT, N], BF16, name=f"xT{k}") for k in range(KD)]
                wv_bf = [wvp.tile([PT, IN], BF16, name=f"wv_bf{k}") for k in range(KD)]
                with tc.tile_pool(name="psP", bufs=1, space="PSUM") as psP:
                    with tc.tile_pool(name="pst", bufs=6, space="PSUM") as pst:
                        # x chunks 0-3 first (they gate the first score
                        # matmuls via xT ch0), then q/k weights, then the rest
                        # of x; wv after x (vproj fills attention(0)); wout last
                        def xchunk(s):
                            st = stage.tile([PT, D], F32, name="st", tag="st")
                            nc.sync.dma_start(st[:], x_ext[s * PT:(s + 1) * PT, :])
                            xbf = xbfp.tile([PT, D], BF16, name="xbf", tag="xbf")
                            nc.vector.tensor_copy(xbf[:], st[:])
                            for k in range(KD):
                                pt_ = pst.tile([PT, PT], BF16, name="pt_", tag="pt")
                                nc.tensor.transpose(
                                    pt_[:], xbf[:, k * PT:(k + 1) * PT], ident[:]
                                )
                                nc.vector.tensor_copy(
                                    xT[k][:, s * PT:(s + 1) * PT], pt_[:]
                                )

                        for s in range(4):
                            xchunk(s)
                        for k in range(KD):
                            for ext, dst in ((wq_ext, wq_bf), (wk_ext, wk_bf)):
                                wst = wstage.tile([PT, IN], F32, name="wst", tag="wst")
                                nc.sync.dma_start(wst[:], ext[k * PT:(k + 1) * PT, :])
                                nc.vector.tensor_copy(dst[k][:], wst[:])
                        for k in range(KD):
                            wst = wstage.tile([PT, IN], F32, name="wst", tag="wst")
                            nc.sync.dma_start(wst[:], wv_ext[k * PT:(k + 1) * PT, :])
                            nc.vector.tensor_copy(wv_bf[k][:], wst[:])
                        for s in range(4, MS):
                            xchunk(s)
                        for k in range(KD):
                            st = stage.tile([PT, D], F32, name="st", tag="st")
                            nc.sync.dma_start(st[:], wout_ext[k * PT:(k + 1) * PT, :])
                            nc.vector.tensor_copy(wo_bf[k][:], st[:])

                    # ============ phases 1+2 interleaved ==============
                    with (
                        tc.tile_pool(name="psS", bufs=2, space="PSUM") as psS,
                        tc.tile_pool(name="psO", bufs=3, space="PSUM") as psO,
                        tc.tile_pool(name="attn", bufs=5) as attnp,
                        tc.tile_pool(name="fin", bufs=2) as finp,
                        tc.tile_pool(name="agst", bufs=2) as agst,
                    ):
                        def vproj_s(s):
                            pv = psP.tile([PT, 512], F32, name="pv", tag="psP")
                            for k in range(KD):
                                nc.tensor.matmul(
                                    pv[:],
                                    xT[k][:, s * PT:(s + 1) * PT],
                                    wv_bf[k][:],
                                    start=(k == 0), stop=(k == KD - 1),
                                )
                            nc.gpsimd.memset(vsb[s][:, :, 64:65], 1.0)
                            nc.vector.tensor_copy(
                                vsb[s][:, :, 0:64],
                                pv[:].rearrange("p (h e) -> p h e", h=GH),
                            )

                        def qkproj(m):
                            for ch in range(4):
                                for w_bf, dstT in ((wq_bf, qT), (wk_bf, kT)):
                                    ph = psP.tile([PT, 512], F32, name="ph", tag="psP")
                                    for k in range(KD):
                                        nc.tensor.matmul(
                                            ph[:],
                                            w_bf[k][:, m * PT:(m + 1) * PT],
                                            xT[k][:, ch * 512:(ch + 1) * 512],
                                            start=(k == 0), stop=(k == KD - 1),
                                        )
                                    nc.vector.tensor_copy(
                                        dstT[m][:, ch * 512:(ch + 1) * 512], ph[:]
                                    )

                        def ag_full(p):
                            nc.sync.dma_start(ag_in[p][:], attoutT[p][:])
                            nc.gpsimd.collective_compute(
                                "AllGather", mybir.AluOpType.bypass,
                                replica_groups=RG,
                                ins=[ag_in[p].opt()], outs=[ag_out[p].opt()],
                            )

                        def ag_iq(p, iq):
                            cs = iq * 512
                            nc.sync.dma_start(
                                ag_cin[iq][:], attoutT[p][:, cs:cs + 512]
                            )
                            nc.gpsimd.collective_compute(
                                "AllGather", mybir.AluOpType.bypass,
                                replica_groups=RG,
                                ins=[ag_cin[iq].opt()],
                                outs=[ag_chunk[iq].opt()],
                            )

                        def attention(p, fill_j=None):
                            last = p == MI - 1
                            # flat software pipeline over (iq, j): v-matmuls
                            # run one step behind S/exp so the next block's
                            # score matmul never queues behind exp-gated work
                            steps = [(iq, j) for iq in range(4) for j in range(MS)]
                            outs = {}
                            pend = None  # (iq, j, at)
                            for iq, j in steps:
                                if j == 0:
                                    outs[iq] = (
                                        psO.tile([65, 512], F32, name="oA", tag="psO"),
                                        psO.tile([65, 512], F32, name="oB", tag="psO"),
                                    )
                                ps = psS.tile([PT, 1024], F32, name="ps", tag="psS")
                                nc.tensor.matmul(
                                    ps[:, 0:512],
                                    kT[p][0:64, j * PT:(j + 1) * PT],
                                    qT[p][0:64, iq * 512:(iq + 1) * 512],
                                    start=True, stop=True,
                                    tile_position=(0, 0),
                                )
                                nc.tensor.matmul(
                                    ps[:, 512:1024],
                                    kT[p][64:128, j * PT:(j + 1) * PT],
                                    qT[p][64:128, iq * 512:(iq + 1) * 512],
                                    start=True, stop=True,
                                    tile_position=(64, 0),
                                )
                                at = attnp.tile([PT, 1024], BF16, name="at", tag="at")
                                nc.scalar.activation(
                                    at[:], ps[:], mybir.ActivationFunctionType.Exp,
                                    scale=SCALE,
                                )
                                if iq == 0 and fill_j is not None:
                                    fill_j(j)
                                if pend is not None:
                                    self_emit_vmm(p, outs, *pend)
                                    if pend[1] == MS - 1:
                                        self_finalize(p, outs, pend[0], last)
                                pend = (iq, j, at)
                            self_emit_vmm(p, outs, *pend)
                            self_finalize(p, outs, pend[0], last)
                            if not last:
                                ag_full(p)
                            # stage + mask-select this pair's two gathered k-tiles
                            for kk in (p, p + MI):
                                half = kk // MI
                                ast = agst.tile([PT, N], BF16, name="ast", tag="ast")
                                if last:
                                    for iq in range(4):
                                        nc.sync.dma_start(
                                            ast[:, iq * 512:(iq + 1) * 512],
                                            ag_chunk[iq][half * PT:(half + 1) * PT, :],
                                        )
                                else:
                                    nc.sync.dma_start(
                                        ast[:], ag_out[p][half * PT:(half + 1) * PT, :]
                                    )
                                tmp = agst.tile([PT, NH], BF16, name="tmp", tag="tmp")
                                nc.vector.tensor_scalar_mul(
                                    tmp[:], ast[:, 0:NH], s0_bc[:]
                                )
                                nc.vector.scalar_tensor_tensor(
                                    attThalf[kk],
                                    ast[:, NH:N], s1_bc[:], tmp[:],
                                    op0=mybir.AluOpType.mult,
                                    op1=mybir.AluOpType.add,
                                )

                        def self_emit_vmm(p, outs, iq, j, at):
                            oA, oB = outs[iq]
                            nc.tensor.matmul(
                                oA[:], vsb[j][:, 2 * p, 0:65], at[:, 0:512],
                                start=(j == 0), stop=(j == MS - 1),
                            )
                            nc.tensor.matmul(
                                oB[:], vsb[j][:, 2 * p + 1, 0:65], at[:, 512:1024],
                                start=(j == 0), stop=(j == MS - 1),
                            )

                        def self_finalize(p, outs, iq, last):
                            dens = []
                            for hh, o in enumerate(outs[iq]):
                                seg = attoutT[p][hh * 64:(hh + 1) * 64,
                                                 iq * 512:(iq + 1) * 512]
                                nc.vector.tensor_copy(seg, o[0:64, :])
                                den = finp.tile([1, 512], F32, name="den", tag="den")
                                nc.vector.tensor_copy(den[:], o[64:65, :])
                                dens.append((hh, den))
                            for hh, den in dens:
                                recip = finp.tile([1, 512], F32, name="recip",
                                                  tag="recip")
                                nc.vector.reciprocal_approx_fast(recip[:], den[:])
                                bc = finp.tile([PT, 512], F32, name="bc", tag="bc")
                                nc.gpsimd.partition_broadcast(bc[:], recip[:])
                                seg = attoutT[p][hh * 64:(hh + 1) * 64,
                                                 iq * 512:(iq + 1) * 512]
                                nc.vector.tensor_tensor(
                                    seg, seg, bc[hh * 64:(hh + 1) * 64, :],
                                    op=mybir.AluOpType.mult,
                                )
                            if last:
                                ag_iq(p, iq)

                        qkproj(0)
                        attention(0, fill_j=vproj_s)
                        for p in range(1, MI):
                            qkproj(p)
                            attention(p)

                # ================= phase 3: output projection ==============
                with (
                    tc.tile_pool(name="pso", bufs=4, space="PSUM") as pso_p,
                    tc.tile_pool(name="osb", bufs=4) as osbp,
                ):
                    korder = [0, 4, 1, 5, 2, 6, 3, 7]
                    for m in range(NH // PT):
                        pso = [
                            pso_p.tile([PT, 512], F32, name="pso", tag="pso")
                            for _ in range(2)
                        ]
                        for nn in range(2):
                            # bias via ones x bias_row (K=1) opens the group
                            nc.tensor.matmul(
                                pso[nn][:], ones_col[:],
                                bias_bf[:, nn * 512:(nn + 1) * 512],
                                start=True, stop=False,
                            )
                        for ki, kk in enumerate(korder):
                            lhs = attThalf[kk][:, m * PT:(m + 1) * PT]
                            for nn in range(2):
                                nc.tensor.matmul(
                                    pso[nn][:],
                                    lhs,
                                    wo_bf[kk][:, nn * 512:(nn + 1) * 512],
                                    start=False, stop=(ki == KD - 1),
                                )
                        for nn in range(2):
                            osb = osbp.tile([PT, 512], F32, name="osb", tag="osb")
                            if nn == 0:
                                nc.scalar.copy(osb[:], pso[nn][:])
                            else:
                                nc.vector.tensor_copy(osb[:], pso[nn][:])
                            nc.sync.dma_start(
                                out_ext[m * PT:(m + 1) * PT, nn * 512:(nn + 1) * 512],
                                osb[:],
                            )

    nc.compile()
    return nc


def _shard_inputs(x, Wq, Wkv, Wout, bout):
    in_maps = []
    for c in range(N_CORES):
        b, g = c // 2, c % 2
        sel = np.zeros((1, 2), dtype=np.float32)
        sel[0, g] = 1.0
        in_maps.append({
            "x": np.ascontiguousarray(x[b], dtype=np.float32),
            "wq": np.ascontiguousarray(Wq[:, g * IN:(g + 1) * IN], dtype=np.float32),
            "wk": np.ascontiguousarray(Wkv[:, g * IN:(g + 1) * IN], dtype=np.float32),
            "wv": np.ascontiguousarray(
                Wkv[:, D + g * IN:D + (g + 1) * IN], dtype=np.float32
            ),
            "wout": np.ascontiguousarray(Wout, dtype=np.float32),
            "bout": np.ascontiguousarray(bout, dtype=np.float32),
            "sel": sel,
        })
    return in_maps


def kernel(x, Wq, Wkv, Wout, bout):
    global _COMPILED
    if _COMPILED is None:
        _COMPILED = build()
    nc = _COMPILED
    in_maps = _shard_inputs(
        np.asarray(x), np.asarray(Wq), np.asarray(Wkv), np.asarray(Wout),
        np.asarray(bout),
    )
    res = bass_utils.run_bass_kernel_spmd(nc, in_maps, core_ids=list(range(N_CORES)))
    out = np.empty((B, N, D), dtype=np.float32)
    for c in range(N_CORES):
        b, g = c // 2, c % 2
        out[b, g * NH:(g + 1) * NH, :] = res.results[c]["out"]
    return out


if __name__ == "__main__":
    rng = np.random.default_rng(0)
    x = rng.standard_normal((B, N, D)).astype(np.float32)
    Wq = rng.standard_normal((D, D)).astype(np.float32) * D ** -0.5
    Wkv = rng.standard_normal((D, 2 * D)).astype(np.float32) * D ** -0.5
    Wout = rng.standard_normal((D, D)).astype(np.float32) * D ** -0.5
    bout = np.zeros((D,), dtype=np.float32)
    y = kernel(x=x, Wq=Wq, Wkv=Wkv, Wout=Wout, bout=bout)
    print("out shape:", y.shape, "finite:", np.isfinite(y).all())

